# revision 27
# baseline (speedup 1.0000x reference)
"""Trainium2 Bass kernel for DifferentiableNeuralGas loss.

loss = mean(exp(-(soft_rank-1)/LAMBDA) * distances) over [N, K]
  distances[n,k] = ||data[n] - weights[k]||_2
  soft_rank[n,i] = 1 + sum_{j != i} sigmoid((d[n,i]-d[n,j])/TAU)

Key identities used on-device:
  S[n,i] := sum_{all j} sigmoid((d_i-d_j)/TAU)   (includes j==i term = 0.5)
  soft_rank - 1 = S - 0.5
  neighborhood = exp(-(S - 0.5)/LAMBDA) = exp(S*(-1/LAMBDA) + 1/(2*LAMBDA))

Triangular packing (USE_TRI): sigma(x)+sigma(-x)=1, so only j>i pairs are
evaluated: 4 full diagonal 32x32 blocks + 6 upper off-diagonal blocks =
10240 sigmoids per 128-row tile instead of 16384 (1.6x less ACT work).
Row-sums of each block feed S[i] directly; column-sums provide the
complementary contributions S[j] += 32 - colsum, with the 32*r constants
folded into per-instruction scalar immediates.

Per-core pipeline (data rows sharded 8 ways, weights replicated):
  A) distances (transposed) D_tT[k,n] = sqrt(w2[k] + x2[n] - 2*w.x):
     fp16 PE matmuls on host-pre-transposed xT + ACT sqrt from PSUM with
     per-partition bias, per 512-block so phase B starts early.
  B) ARG = d_i - d_j via one fp16 PE matmul per 512-slice against the
     constant triangular selection matrix msel; 2048-wide ACT sigmoid
     instructions (PSUM->SBUF fp16); j/i-sums via fp16 fold trees (DVE 2x
     packed mode, strided APs) + f32 finals; S assembled from row/col
     sums with scalar_tensor_tensor immediates.
  C) split ACT exp, 16 PE transposes into one PSUM slab, single fused
     multiply+accum -> per-partition partials DMA'd out.
Host sums the 8x128 partials and divides by N*K.

Measured: ~205us HW exec, loss rel err ~3.2e-5. ACT streams sigmoids at
100% during the main phase (1858ns cadence, zero gaps); DVE at ~94%.
"""

import sys

sys.path.insert(0, "/opt/trn_rl_repo")

from contextlib import ExitStack

import numpy as np

import concourse.bass as bass
import concourse.mybir as mybir
import concourse.tile as tile
from concourse import bacc
from concourse.bass_utils import run_bass_kernel_spmd


def _install_ntff_hook():
    """The agent image's antenv lacks axon_hooks, so trn_boot's NTFF
    profile hook never registers; recreate the tiny registry here so
    trace=True can capture HW profiles through libaxon_pjrt."""
    import types

    if "antenv.axon_hooks" in sys.modules:
        return
    mod = types.ModuleType("antenv.axon_hooks")
    _hook = [None]
    mod.set_axon_ntff_profile_hook = lambda h: _hook.__setitem__(0, h)
    mod.get_axon_ntff_profile_hook = lambda: _hook[0]
    sys.modules["antenv.axon_hooks"] = mod
    try:
        import trn_agent_boot.trn_boot as tb

        mod.set_axon_ntff_profile_hook(
            tb._ntff_profile_via_ctypes("/opt/axon/libaxon_pjrt.so"))
    except Exception:
        pass


_install_ntff_hook()

F32 = mybir.dt.float32
F16 = mybir.dt.float16
AF = mybir.ActivationFunctionType
ALU = mybir.AluOpType
AX = mybir.AxisListType

N, D, K = 16384, 64, 128
NCORES = 8
TAU = 0.2
LAMBDA = 8.0
P = 128
CH = 2048            # free-dim elements per sigmoid chunk
MM = 512             # fp32 moving-operand max per matmul
GRP = CH // K        # i-values per chunk
USE_HILO = False     # exact fp16 hi/lo ARG matmuls (2x PE) vs single fp16
USE_TRI = True       # triangular packing: sigma(x)+sigma(-x)=1 halves ACT
BS = 32              # triangle block size
NRB = K // BS        # 4 row blocks
OFFB = [(0, 1), (0, 2), (1, 2), (0, 3), (1, 3), (2, 3)]  # by 2nd coord
TRI = NRB * BS * BS + len(OFFB) * BS * BS  # 4096 + 6144 = 10240


def build(nloc: int) -> bass.Bass:
    nt = nloc // P
    nch = (K * K) // CH

    nc = bacc.Bacc()
    xT_d = nc.dram_tensor("xT", [D, nloc], F16, kind="ExternalInput")
    wTm2_d = nc.dram_tensor("wTm2", [D, K], F16, kind="ExternalInput")
    w2col_d = nc.dram_tensor("w2col", [K, 1], F32, kind="ExternalInput")
    mselw = TRI if USE_TRI else K * K
    msel_d = nc.dram_tensor("msel", [K, mselw], F16, kind="ExternalInput")
    out_d = nc.dram_tensor("out", [P, 2], F32, kind="ExternalOutput")

    with ExitStack() as ctx:
        tc = ctx.enter_context(tile.TileContext(nc))
        singles = ctx.enter_context(tc.tile_pool(name="singles", bufs=1))

        wT_m2 = singles.tile([D, K], F16, tag="wTm2")
        nc.sync.dma_start(out=wT_m2, in_=wTm2_d[:, :])
        w2col = singles.tile([K, 1], F32, tag="w2col")
        nc.sync.dma_start(out=w2col, in_=w2col_d[:, :])
        ones64 = singles.tile([D, P], F16, tag="ones64")
        nc.vector.memset(ones64, 1.0)
        expbias = singles.tile([P, 1], F32, tag="expbias")
        nc.vector.memset(expbias, 1.0 / (2.0 * LAMBDA))

        dtt = {}   # t -> D_tT tile [K=128 part (cluster), P free (row)]
        dhi = {}
        dlo = {}
        S_tiles = {}

        # ---------------- phase A: distances ----------------
        D_all = singles.tile([K, nt * P], F16, tag="D_all")
        with tc.tile_pool(name="psumA", bufs=2, space="PSUM") as psumA:
            xT_all = singles.tile([D, nloc], F16, tag="xT_all")
            BB = min(512, nloc)
            for b in range(nloc // BB):
                nc.sync.dma_start(out=xT_all[:, b * BB:(b + 1) * BB],
                                  in_=xT_d[:, b * BB:(b + 1) * BB])
            xsq_all = singles.tile([D, nloc], F16, tag="xsq_all")
            for b in range(nloc // BB):
                sl = slice(b * BB, (b + 1) * BB)
                nc.vector.scalar_tensor_tensor(
                    out=xsq_all[:, sl], in0=xT_all[:, sl], scalar=1.0,
                    in1=xT_all[:, sl], op0=ALU.bypass, op1=ALU.mult)
                psum_dT = psumA.tile([K, BB], F32, tag="dT")
                nc.tensor.matmul(psum_dT, wT_m2, xT_all[:, sl],
                                 start=True, stop=False)
                nc.tensor.matmul(psum_dT, ones64, xsq_all[:, sl],
                                 start=False, stop=True)
                # sqrt straight from PSUM + fp16 cast, per block, so
                # phase B can start as soon as possible
                nc.scalar.activation(D_all[:, sl], psum_dT, AF.Sqrt,
                                     bias=w2col, scale=1.0)
        for t in range(nt):
            dtt[t] = D_all[:, t * P:(t + 1) * P]
            dhi[t] = dtt[t]
        if USE_HILO:
            Dlo_all = singles.tile([K, nt * P], F16, tag="Dlo_all")
            nc.vector.scalar_tensor_tensor(
                out=Dlo_all, in0=D_all, scalar=1.0, in1=Dhi_all,
                op0=ALU.bypass, op1=ALU.subtract)
            for t in range(nt):
                dlo[t] = Dlo_all[:, t * P:(t + 1) * P]

        # msel DMAs issued after phase A's so x tiles aren't queued
        # behind MBs of constants (they finish well before phase B needs them)
        nch_eff = mselw // CH
        msel_sb = []
        for c in range(nch_eff):
            m = singles.tile([P, CH], F16, tag=f"msel{c}")
            nc.sync.dma_start(out=m, in_=msel_d[:, c * CH:(c + 1) * CH])
            msel_sb.append(m)

        def msel_slice(pos, width):
            c0 = pos // CH
            assert pos % MM == 0 and c0 == (pos + width - 1) // CH
            return msel_sb[c0][:, pos - c0 * CH:pos - c0 * CH + width]

        # D row-major for phase C via DMA xbar transposes: runs on the
        # (otherwise idle) DMA engines during phase B, no PE/PSUM use
        Dt = singles.tile([P, nt * K], F16, tag="Dt")
        for t in range(nt):
            nc.sync.dma_start_transpose(
                out=Dt[:, t * K:(t + 1) * K], in_=dtt[t])

        # ---------------- phase B (triangular variant) ----------------
        if USE_TRI:
            with tc.tile_pool(name="psumB", bufs=2, space="PSUM") as psumB, \
                 tc.tile_pool(name="sigp", bufs=3) as sigp, \
                 tc.tile_pool(name="foldp", bufs=2) as foldp:
                S_all = singles.tile([P, nt * K], F32, tag="S_all")
                nbl = NRB + len(OFFB)           # 10 blocks
                for t in range(nt):
                    S_t = S_all[:, t * K:(t + 1) * K]
                    S_tiles[t] = S_t
                    sig = sigp.tile([P, TRI], F16, tag="sigtri")
                    # merge TTs go to GPSIMD except on the last tile
                    meng = nc.gpsimd if t < nt - 1 else nc.vector
                    for c in range(TRI // CH):
                        psum_arg = psumB.tile([P, CH], F32, tag="arg")
                        for m in range(CH // MM):
                            nc.tensor.matmul(
                                psum_arg[:, m * MM:(m + 1) * MM],
                                dhi[t],
                                msel_slice(c * CH + m * MM, MM),
                                start=True, stop=True,
                                skip_group_check=True)
                        nc.scalar.activation(
                            sig[:, c * CH:(c + 1) * CH], psum_arg,
                            AF.Sigmoid, bias=0.0, scale=1.0 / TAU)
                    # row-sums over j (fp16 2x folds); batched for most
                    # tiles, per-chunk for the last tile to keep the
                    # kernel tail short
                    rowsums = foldp.tile([P, nbl * BS], F16, tag="rows")
                    colsums = foldp.tile([P, len(OFFB) * BS], F16,
                                         tag="cols")

                    def rowfold(view, nrows, out_sl, tagsfx):
                        cur = view
                        jw = BS
                        while jw > 2:
                            nxt = foldp.tile([P, nrows * jw // 2], F16,
                                             tag=f"rf{jw}{tagsfx}")
                            nc.vector.tensor_tensor(
                                out=nxt[:].rearrange(
                                    "p (bi j) -> p bi j", j=jw // 2),
                                in0=cur[:, :, 0:jw // 2],
                                in1=cur[:, :, jw // 2:jw], op=ALU.add)
                            cur = nxt[:].rearrange("p (bi j) -> p bi j",
                                                   j=jw // 2)
                            jw //= 2
                        nc.vector.tensor_tensor(
                            out=rowsums[:, out_sl].rearrange(
                                "p (bi j) -> p bi j", j=1),
                            in0=cur[:, :, 0:1], in1=cur[:, :, 1:2],
                            op=ALU.add)

                    def colfold(view, nb, out_sl, tagsfx):
                        curc = view
                        iw = BS
                        while iw > 2:
                            nxtc = foldp.tile(
                                [P, nb * (iw // 2) * BS], F16,
                                tag=f"cf{iw}{tagsfx}")
                            nc.vector.tensor_tensor(
                                out=nxtc[:].rearrange(
                                    "p (b i j) -> p b i j",
                                    i=iw // 2, j=BS),
                                in0=curc[:, :, 0:iw // 2, :],
                                in1=curc[:, :, iw // 2:iw, :], op=ALU.add)
                            curc = nxtc[:].rearrange(
                                "p (b i j) -> p b i j", i=iw // 2, j=BS)
                            iw //= 2
                        nc.vector.tensor_tensor(
                            out=colsums[:, out_sl].rearrange(
                                "p (b i j) -> p b i j", i=1, j=BS),
                            in0=curc[:, :, 0:1, :], in1=curc[:, :, 1:2, :],
                            op=ALU.add)

                    rowfold(sig[:].rearrange("p (bi j) -> p bi j",
                                             j=BS),
                            nbl * BS, slice(0, nbl * BS), "")
                    colfold(sig[:, NRB * BS * BS:TRI].rearrange(
                                "p (b i j) -> p b i j", i=BS, j=BS),
                            len(OFFB), slice(0, len(OFFB) * BS), "")

                    # diag blocks 0..3 at rowsums[0..3]; OFFB at 4..9
                    # fp16 partial merges (2x mode); final op per r-group
                    # writes f32 into S
                    def rsl(b):
                        return rowsums[:, b * BS:(b + 1) * BS]

                    def ssl(r):
                        return S_t[:, r * BS:(r + 1) * BS]
                    stmp = foldp.tile([P, K], F16, tag="stmp")
                    meng.tensor_tensor(out=stmp[:, 0:BS], in0=rsl(0),
                                       in1=rsl(4), op=ALU.add)
                    meng.tensor_tensor(out=stmp[:, 0:BS],
                                       in0=stmp[:, 0:BS],
                                       in1=rsl(5), op=ALU.add)
                    meng.tensor_tensor(out=ssl(0),
                                       in0=stmp[:, 0:BS],
                                       in1=rsl(7), op=ALU.add)
                    meng.tensor_tensor(out=stmp[:, BS:2 * BS],
                                       in0=rsl(1), in1=rsl(6),
                                       op=ALU.add)
                    meng.tensor_tensor(out=stmp[:, BS:2 * BS],
                                       in0=stmp[:, BS:2 * BS],
                                       in1=rsl(8), op=ALU.add)
                    meng.tensor_tensor(out=stmp[:, 2 * BS:3 * BS],
                                       in0=rsl(2), in1=rsl(9),
                                       op=ALU.add)
                    # colsum groups by second coord: R=1:{b0} R=2:{b1,b2}
                    # R=3:{b3,b4,b5} (colsums index = OFFB order)
                    c2 = foldp.tile([P, BS], F16, tag="c2")
                    meng.tensor_tensor(
                        out=c2, in0=colsums[:, BS:2 * BS],
                        in1=colsums[:, 2 * BS:3 * BS], op=ALU.add)
                    c3 = foldp.tile([P, BS], F16, tag="c3")
                    meng.tensor_tensor(
                        out=c3, in0=colsums[:, 3 * BS:4 * BS],
                        in1=colsums[:, 4 * BS:5 * BS], op=ALU.add)
                    meng.tensor_tensor(
                        out=c3, in0=c3, in1=colsums[:, 5 * BS:6 * BS],
                        op=ALU.add)
                    nc.vector.scalar_tensor_tensor(
                        out=ssl(1), in0=stmp[:, BS:2 * BS],
                        scalar=float(BS), in1=colsums[:, 0:BS],
                        op0=ALU.add, op1=ALU.subtract)
                    nc.vector.scalar_tensor_tensor(
                        out=ssl(2), in0=stmp[:, 2 * BS:3 * BS],
                        scalar=float(2 * BS), in1=c2,
                        op0=ALU.add, op1=ALU.subtract)
                    nc.vector.scalar_tensor_tensor(
                        out=ssl(3), in0=rsl(3), scalar=float(3 * BS),
                        in1=c3, op0=ALU.add, op1=ALU.subtract)

        # ---------------- phase B: sigmoid rank sums ----------------
        if USE_TRI:
            nch = 0  # full-matrix path skipped
        with tc.tile_pool(name="psumB", bufs=2, space="PSUM") as psumB, \
             tc.tile_pool(name="sigp", bufs=4) as sigp, \
             tc.tile_pool(name="foldp", bufs=3) as foldp:
            if not USE_TRI:
                S_all = singles.tile([P, nt * K], F32, tag="S_all")
            for t in range(nt if not USE_TRI else 0):
                S_t = S_all[:, t * K:(t + 1) * K]
                S_tiles[t] = S_t
                f1 = foldp.tile([P, K * K // 2], F16, tag="f1")
                for c in range(nch):
                    psum_arg = psumB.tile([P, CH], F32, tag="arg")
                    nmm = CH // MM
                    for m in range(nmm):
                        nc.tensor.matmul(
                            psum_arg[:, m * MM:(m + 1) * MM],
                            dhi[t],
                            msel_sb[c][:, m * MM:(m + 1) * MM],
                            start=True, stop=not USE_HILO,
                            skip_group_check=True)
                    if USE_HILO:
                        for m in range(nmm):
                            nc.tensor.matmul(
                                psum_arg[:, m * MM:(m + 1) * MM],
                                dlo[t],
                                msel_sb[c][:, m * MM:(m + 1) * MM],
                                start=False, stop=True, skip_group_check=True)
                    sig = sigp.tile([P, CH], F16, tag="sig")
                    nc.scalar.activation(sig, psum_arg, AF.Sigmoid,
                                         bias=0.0, scale=1.0 / TAU)
                    if t < nt - 1:
                        # per-chunk fold1 (fp16 2x); rest at tile level
                        s3 = sig[:].rearrange("p (i j) -> p i j", j=K)
                        nc.vector.tensor_tensor(
                            out=f1[:, c * CH // 2:(c + 1) * CH // 2]
                                .rearrange("p (i j) -> p i j", j=K // 2),
                            in0=s3[:, :, 0:K // 2], in1=s3[:, :, K // 2:K],
                            op=ALU.add)
                    else:
                        # last tile: fold all the way per chunk so the
                        # kernel tail isn't serialized on a deep fold tree
                        cur = sig
                        jw = K
                        while jw > 4:
                            nxt = foldp.tile([P, GRP * jw // 2], F16,
                                             tag=f"fc{jw}")
                            cv = cur[:].rearrange("p (i j) -> p i j", j=jw)
                            nc.vector.tensor_tensor(
                                out=nxt[:].rearrange("p (i j) -> p i j",
                                                     j=jw // 2),
                                in0=cv[:, :, 0:jw // 2],
                                in1=cv[:, :, jw // 2:jw],
                                op=ALU.add)
                            cur = nxt
                            jw //= 2
                        nc.vector.reduce_sum(
                            out=S_t[:, c * GRP:(c + 1) * GRP],
                            in_=cur[:].rearrange("p (i j) -> p i j", j=jw),
                            axis=AX.X)
                if t < nt - 1:
                    f1v = f1[:].rearrange("p (i j) -> p i j", j=K // 2)
                    f2 = foldp.tile([P, K * K // 4], F16, tag="f2")
                    nc.vector.tensor_tensor(
                        out=f2[:].rearrange("p (i j) -> p i j", j=K // 4),
                        in0=f1v[:, :, 0:K // 4],
                        in1=f1v[:, :, K // 4:K // 2], op=ALU.add)
                    f2v = f2[:].rearrange("p (i j) -> p i j", j=K // 4)
                    f3 = foldp.tile([P, K * K // 8], F16, tag="f3")
                    nc.vector.tensor_tensor(
                        out=f3[:].rearrange("p (i j) -> p i j", j=K // 8),
                        in0=f2v[:, :, 0:K // 8],
                        in1=f2v[:, :, K // 8:K // 4], op=ALU.add)
                    nc.vector.reduce_sum(
                        out=S_t,
                        in_=f3[:].rearrange("p (i j) -> p i j", j=K // 8),
                        axis=AX.X)

        # ---------------- phase C: neighborhood * distance ----------------
        # exp+mult for tiles 0..nt-2 are emitted after ALL sigmoids in
        # ACT program order (one table switch) and execute under the DVE
        # fold overhang; only the last tile's exp+mult sit in the tail.
        # The multiply runs as tensor_tensor (fp16 2x) + a tensor_scalar
        # accumulation pass (4x) instead of a 1x scalar_tensor_tensor.
        E_all = singles.tile([P, nt * K], F16, tag="E_all")
        scr = singles.tile([P, nt * K], F16, tag="scrC")
        scrb = singles.tile([P, nt * K], F16, tag="scrD")
        loss2 = singles.tile([P, 2], F32, tag="loss2")
        n14 = (nt - 1) * K
        nc.scalar.activation(E_all[:, 0:n14], S_all[:, 0:n14], AF.Exp,
                             bias=expbias, scale=-1.0 / LAMBDA)
        nc.vector.tensor_tensor(out=scr[:, 0:n14], in0=E_all[:, 0:n14],
                                in1=Dt[:, 0:n14], op=ALU.mult)
        nc.vector.tensor_scalar(out=scrb[:, 0:n14], in0=scr[:, 0:n14],
                                scalar1=1.0, scalar2=0.0, op0=ALU.mult,
                                op1=ALU.add, accum_out=loss2[:, 0:1])
        nc.scalar.activation(E_all[:, n14:], S_all[:, n14:], AF.Exp,
                             bias=expbias, scale=-1.0 / LAMBDA)
        nc.vector.tensor_tensor(out=scr[:, n14:], in0=E_all[:, n14:],
                                in1=Dt[:, n14:], op=ALU.mult)
        nc.vector.tensor_scalar(out=scrb[:, n14:], in0=scr[:, n14:],
                                scalar1=1.0, scalar2=0.0, op0=ALU.mult,
                                op1=ALU.add, accum_out=loss2[:, 1:2])
        nc.sync.dma_start(out=out_d[:, :], in_=loss2)

    nc.finalize()
    return nc


def make_msel() -> np.ndarray:
    I = np.eye(K, dtype=np.float32)
    if not USE_TRI:
        return np.ascontiguousarray(
            (I[:, :, None] - I[:, None, :]).reshape(K, K * K)
            .astype(np.float16))
    cols = []
    for r in range(NRB):
        a = I[:, r * BS:(r + 1) * BS]
        cols.append((a[:, :, None] - a[:, None, :]).reshape(K, BS * BS))
    for (r, cc) in OFFB:
        a = I[:, r * BS:(r + 1) * BS]
        b = I[:, cc * BS:(cc + 1) * BS]
        cols.append((a[:, :, None] - b[:, None, :]).reshape(K, BS * BS))
    return np.ascontiguousarray(
        np.concatenate(cols, axis=1).astype(np.float16))


_BUILT: dict[int, bass.Bass] = {}


def get_built(nloc: int) -> bass.Bass:
    if nloc not in _BUILT:
        _BUILT[nloc] = build(nloc)
    return _BUILT[nloc]


def make_in_maps(data: np.ndarray, weights: np.ndarray, ncores: int):
    nloc = data.shape[0] // ncores
    msel = make_msel()
    wTm2 = np.ascontiguousarray((-2.0 * weights.T).astype(np.float16))
    w64 = weights.astype(np.float64)
    w2col = np.ascontiguousarray(
        (w64 * w64).sum(axis=1).astype(np.float32).reshape(K, 1))
    return [
        {
            "xT": np.ascontiguousarray(
                data[c * nloc:(c + 1) * nloc].T.astype(np.float16)),
            "wTm2": wTm2,
            "w2col": w2col,
            "msel": msel,
        }
        for c in range(ncores)
    ]


def run(data, weights, trace: bool = False):
    """Returns (loss, BassKernelResults)."""
    data = np.ascontiguousarray(np.asarray(data, dtype=np.float32))
    weights = np.ascontiguousarray(np.asarray(weights, dtype=np.float32))
    n, k = data.shape[0], weights.shape[0]
    nloc = n // NCORES
    nc = get_built(nloc)
    in_maps = make_in_maps(data, weights, NCORES)
    res = run_bass_kernel_spmd(nc, in_maps, list(range(NCORES)), trace=trace)
    total = sum(float(r["out"].sum(dtype=np.float64)) for r in res.results)
    loss = np.float32(total / (n * k))
    return loss, res


def kernel(data, weights):
    loss, _ = run(data, weights)
    return loss

